# revision 11
# baseline (speedup 1.0000x reference)
"""Local causal (sliding-window) attention kernel for Trainium2, SPMD over 8 cores.

Problem: states [4, 4096, 1024] f32; q/k/v = states @ W*.T + b*; each query t
attends keys t-8..t (window=8), softmax over valid positions, out = attn @ v.

Sharding: data-parallel, 8 shards = 4 batches x 2 sequence halves (2048 queries
each). The host supplies each shard's states pre-transposed to [H, 2056] with an
8-row halo (zero-padded at sequence start; masked out via the additive mask).

Score reformulation (saves one full GEMM): q.k = x_t^T A x_k + g.x_t + w.x_k
+ c0 with A = (Wq/sqrt(H))^T Wk precomputed on host. The device computes
Y = A @ X (one GEMM) instead of both Q and K projections; X itself serves as
the score lhsT. The rank-1 term u[k] = w.x_k is a host GEMV accumulated into
the score PSUM via a tiny K=1 matmul; per-query terms/constants cancel in
softmax, so no max-subtraction is needed (scores are O(1), masked lanes get
-30000 and underflow to exact zero in exp).

Scheduling notes (from perfetto traces): there is a ~7us fixed program
preamble before the first DMA descriptor can issue, each DMA descriptor
costs ~630ns on its issuing queue, and completion semaphores batch per
queue, so the input phase is descriptor-issue-bound, not bandwidth-bound.
Hence: few large DMAs (consts packed, Wv/A as single wide tiles loaded in 2
DMAs), issues spread over 4 engine queues, warm-up matmuls on a memset tile
to keep the PE HAM-warm through the load phase, V-GEMM groups emitted before
Y so the earliest-landing data is consumed first, bf16 output DMA.
"""

import numpy as np
import ml_dtypes

import concourse.bacc as bacc
import concourse.mybir as mybir
import concourse.tile as tile
from concourse.bass_utils import run_bass_kernel_spmd

B, T, H = 4, 4096, 1024
NCORES = 8
TC = T // 2            # queries per core
HALO = 8               # window size
TH = TC + HALO         # shard cols incl. halo
SPAN = 128 + HALO      # key span per 128-query tile
NT = TC // 128         # query tiles per core
HC = H // 128          # 128-row chunks of H
F32 = mybir.dt.float32
BF16 = mybir.dt.bfloat16
BF = ml_dtypes.bfloat16
AF = mybir.ActivationFunctionType
NWARM = 24             # HAM warm-up matmuls during the DMA load phase

_cache = {}


def _emit(nc, tc, aps, pools):
    (x_d, a_d, wv_d, cf_d, cb_d, uo_d, yh_d, vt_d, out_d) = aps
    consts, xw, acts, psP, psS, psT, psO, attn = pools

    # packed f32 consts: [bv | m0 | mr]
    cf = consts.tile([128, H + 2 * SPAN], F32, tag="cf", name="cf")
    bv_t = cf[:, 0:H]
    m0_t = cf[:, H:H + SPAN]
    mr_t = cf[:, H + SPAN:H + 2 * SPAN]
    id_t = consts.tile([128, 128], BF16, tag="id", name="id_t")
    uo = consts.tile([1, 128 + TH], BF16, tag="uo", name="uo")
    on_t = uo[:, 0:128]
    u_t = uo[:, 128:128 + TH]
    warm = consts.tile([128, 512], BF16, tag="warm", name="warm")

    xt = [xw.tile([128, TH], BF16, tag=f"x{c}", name=f"x{c}") for c in range(HC)]
    a_all = xw.tile([128, HC * H], BF16, tag="a_all", name="a_all")
    wv_all = xw.tile([128, HC * H], BF16, tag="wv_all", name="wv_all")
    at = [a_all[:, c * H:(c + 1) * H] for c in range(HC)]
    wvt = [wv_all[:, c * H:(c + 1) * H] for c in range(HC)]
    yt = [acts.tile([128, TH], BF16, tag=f"y{c}", name=f"y{c}") for c in range(HC)]
    vt = [acts.tile([128, H], BF16, tag=f"v{j}", name=f"v{j}")
          for j in range(NT)]
    vtail = acts.tile([HALO, H], BF16, tag="vtail", name="vtail")

    # --- DMA issue plan: 4 queues, few large descriptors, FIFO per queue =
    # need order. scalar/vector queues are idle early, use them for consts
    # and the first x segment.
    nc.gpsimd.memset(warm[:], 1.0)
    nc.scalar.dma_start(cf[:], cf_d[:])
    nc.scalar.dma_start(id_t[:], cb_d[:])
    nc.scalar.dma_start(uo[:], uo_d[:])
    # wv first (V group 0 consumes it first), halves on two queues
    nc.gpsimd.dma_start(wv_all[:, 0:4 * H], wv_d[:, 0:4 * H])
    nc.sync.dma_start(wv_all[:, 4 * H:8 * H], wv_d[:, 4 * H:8 * H])
    # x segment 0 (cols 0:520) per chunk, alternating queues
    for c in range(HC):
        (nc.scalar if c % 2 else nc.sync).dma_start(
            xt[c][:, 0:520], x_d[c * 128:(c + 1) * 128, 0:520])
    # A halves
    nc.gpsimd.dma_start(a_all[:, 0:4 * H], a_d[:, 0:4 * H])
    nc.sync.dma_start(a_all[:, 4 * H:8 * H], a_d[:, 4 * H:8 * H])
    # Y halo cols + vtail (needed by attention 0 / 15)
    for c in range(HC):
        nc.scalar.dma_start(yt[c][:, 0:HALO], yh_d[c * 128:(c + 1) * 128, :])
    nc.scalar.dma_start(vtail[:], vt_d[:])
    # remaining x segments
    for seg in range(3):
        lo = 520 + seg * 512
        for c in range(HC):
            q = (nc.gpsimd, nc.sync, nc.scalar)[c % 3]
            q.dma_start(xt[c][:, lo:lo + 512],
                        x_d[c * 128:(c + 1) * 128, lo:lo + 512])

    # --- HAM warm-up: junk matmuls on the memset tile fill the PE during
    # the load phase so real matmuls start at full clock.
    for w in range(NWARM):
        ps = psP.tile([128, 512], F32, tag="ps", name="pswarm")
        nc.tensor.matmul(ps[:], warm[:, 0:128], warm[:], start=True, stop=True)

    def emit_y(t4):
        off = HALO + t4 * 512
        for hc in range(HC):
            ps = psP.tile([128, 512], F32, tag="ps", name="psy")
            for c in range(HC):
                nc.tensor.matmul(
                    ps[:], at[c][:, hc * 128:(hc + 1) * 128],
                    xt[c][:, off: off + 512],
                    start=(c == 0), stop=(c == HC - 1))
            # alternate copy engine to balance Scalar/Vector queues
            if hc % 2 == 0:
                nc.scalar.copy(yt[hc][:, off: off + 512], ps[:])
            else:
                nc.vector.tensor_copy(yt[hc][:, off: off + 512], ps[:])

    def emit_v(j):
        for hh in range(2):
            ps = psP.tile([128, 512], F32, tag="ps", name="psv")
            for c in range(HC):
                nc.tensor.matmul(
                    ps[:], xt[c][:, j * 128: (j + 1) * 128],
                    wvt[c][:, hh * 512:(hh + 1) * 512],
                    start=(c == 0), stop=(c == HC - 1))
            nc.vector.tensor_add(
                vt[j][:, hh * 512:(hh + 1) * 512], ps[:],
                bv_t[:, hh * 512:(hh + 1) * 512])

    def emit_attn(j):
        s_ps = psS.tile([128, SPAN], F32, tag="s", name="s_ps")
        # rank-1 u term first: its LDWEIGHTS prefetches under prior streams
        nc.tensor.matmul(s_ps[:], on_t[:, 0:128],
                         u_t[:, j * 128: j * 128 + SPAN],
                         start=True, stop=False)
        for c in range(HC):
            nc.tensor.matmul(
                s_ps[:], xt[c][:, HALO + j * 128: HALO + (j + 1) * 128],
                yt[c][:, j * 128: j * 128 + SPAN],
                start=False, stop=(c == HC - 1))
        s_sb = attn.tile([128, SPAN], F32, tag="ssb", name="s_sb")
        nc.vector.tensor_add(s_sb[:], s_ps[:],
                             (m0_t if j == 0 else mr_t)[:])
        p_bf = attn.tile([128, SPAN], BF16, tag="p", name="p_bf")
        rowsum = attn.tile([128, 1], F32, tag="rs", name="rowsum")
        nc.scalar.activation(p_bf[:], s_sb[:], AF.Exp,
                             bias=0.0, scale=1.0,
                             accum_out=rowsum[:])
        rinv = attn.tile([128, 1], F32, tag="ri", name="rinv")
        nc.vector.reciprocal(rinv[:], rowsum[:])

        pt_ps = psT.tile([128, 256], BF16, tag="pt", name="pt_ps")
        nc.tensor.transpose(pt_ps[:, 0:128], p_bf[:, 0:128], id_t[:])
        nc.tensor.transpose(pt_ps[:HALO, 128:256], p_bf[:, 128:SPAN], id_t[:])
        ptab = attn.tile([128, 256], BF16, tag="ptab", name="ptab")
        nc.scalar.copy(ptab[:], pt_ps[:])

        vnext = vtail if j == NT - 1 else vt[j + 1]
        out_sb = attn.tile([128, H], BF16, tag="osb", name="out_sb")
        for hh in range(2):
            o_ps = psO.tile([128, 512], F32, tag="o", name="o_ps")
            nc.tensor.matmul(o_ps[:], ptab[:, 0:128],
                             vt[j][:, hh * 512:(hh + 1) * 512],
                             start=True, stop=False)
            nc.tensor.matmul(o_ps[:], ptab[:HALO, 128:256],
                             vnext[:HALO, hh * 512:(hh + 1) * 512],
                             start=False, stop=True)
            nc.scalar.activation(
                out_sb[:, hh * 512:(hh + 1) * 512], o_ps[:],
                AF.Copy, bias=0.0, scale=rinv[:])
        nc.sync.dma_start(out_d[j * 128:(j + 1) * 128, :], out_sb[:])

    # Interleave: V group first (its data lands first), then Y segment,
    # then attention tiles whose span fits the Y columns produced so far.
    # attn j needs Y cols < 8+512*(t4+1) and vt[j+1] (so attn(4*t4-1) waits
    # for the following V group).
    for t4 in range(TC // 512):
        for j in range(4 * t4, 4 * t4 + 4):
            emit_v(j)
        if t4 > 0:
            emit_attn(4 * t4 - 1)
        emit_y(t4)
        for j in range(4 * t4, 4 * t4 + 3):
            emit_attn(j)
    emit_attn(NT - 1)


def _build(loop_reps=None, trace_sim=False):
    key = ("nc", loop_reps, trace_sim)
    if key in _cache:
        return _cache[key]
    nc = bacc.Bacc("TRN2", target_bir_lowering=False, debug=False,
                   num_devices=NCORES)

    aps = (
        nc.dram_tensor("x", [H, TH], BF16, kind="ExternalInput").ap(),
        nc.dram_tensor("a", [128, HC * H], BF16, kind="ExternalInput").ap(),
        nc.dram_tensor("wv", [128, HC * H], BF16, kind="ExternalInput").ap(),
        nc.dram_tensor("cf", [128, H + 2 * SPAN], F32,
                       kind="ExternalInput").ap(),
        nc.dram_tensor("ident", [128, 128], BF16, kind="ExternalInput").ap(),
        nc.dram_tensor("uo", [1, 128 + TH], BF16, kind="ExternalInput").ap(),
        nc.dram_tensor("yhalo", [H, HALO], BF16, kind="ExternalInput").ap(),
        nc.dram_tensor("vtail", [HALO, H], BF16, kind="ExternalInput").ap(),
        nc.dram_tensor("out", [TC, H], BF16, kind="ExternalOutput").ap(),
    )

    with tile.TileContext(nc, trace_sim=trace_sim) as tc:
        with (
            tc.tile_pool(name="consts", bufs=1) as consts,
            tc.tile_pool(name="xw", bufs=1) as xw,
            tc.tile_pool(name="acts", bufs=1) as acts,
            tc.tile_pool(name="psP", bufs=2, space="PSUM") as psP,
            tc.tile_pool(name="psS", bufs=2, space="PSUM") as psS,
            tc.tile_pool(name="psT", bufs=2, space="PSUM") as psT,
            tc.tile_pool(name="psO", bufs=2, space="PSUM") as psO,
            tc.tile_pool(name="attn", bufs=3) as attn,
        ):
            pools = (consts, xw, acts, psP, psS, psT, psO, attn)
            if loop_reps:
                with tc.For_i(0, loop_reps, 1):
                    _emit(nc, tc, aps, pools)
            else:
                _emit(nc, tc, aps, pools)

    nc.compile()
    _cache[key] = nc
    return nc


def _host_inputs(states, Wq, bq, Wk, bk, Wv, bv):
    """Shared (per-run) host-side tensor prep."""
    scale = 1.0 / np.sqrt(H)
    Wq = np.asarray(Wq, np.float32)
    Wk = np.asarray(Wk, np.float32)
    Wv = np.asarray(Wv, np.float32)
    bq = np.asarray(bq, np.float32)
    bk = np.asarray(bk, np.float32)
    bv = np.asarray(bv, np.float32)
    Wqs = Wq * scale
    # A = Wqs.T @ Wk ; device lhsT layout needs A.T = Wk.T @ Wqs
    at_h = np.ascontiguousarray(Wk.T @ Wqs).astype(BF)
    # per-key rank-1 vector; per-query term and constants cancel in softmax
    wt_h = Wk.T @ (bq * scale)
    wv_h = np.ascontiguousarray(Wv.T).astype(BF)
    # chunk-packed [128, 8*H] device layouts: block c = rows c*128..c*128+127
    at_p = np.ascontiguousarray(
        at_h.reshape(HC, 128, H).transpose(1, 0, 2).reshape(128, HC * H))
    wv_p = np.ascontiguousarray(
        wv_h.reshape(HC, 128, H).transpose(1, 0, 2).reshape(128, HC * H))
    m = np.arange(128)[:, None]
    n = np.arange(SPAN)[None, :]
    band = (n >= m) & (n <= m + HALO)
    mr_h = np.where(band, 0.0, -30000.0).astype(np.float32)
    m0_h = np.where(band & (n >= HALO), 0.0, -30000.0).astype(np.float32)
    id_h = np.eye(128).astype(BF)
    return at_h, at_p, wt_h, wv_h, wv_p, m0_h, mr_h, id_h, bv


def _shard_maps(states, hosts):
    at_h, at_p, wt_h, wv_h, wv_p, m0_h, mr_h, id_h, bv = hosts
    a_f = at_h.astype(np.float32)      # [hin, hout] = A.T in bf16 precision
    wv_f = wv_h.astype(np.float32)
    bv_b = np.broadcast_to(bv, (128, H))
    cf0 = np.ascontiguousarray(
        np.concatenate([bv_b, m0_h, mr_h], axis=1)).astype(np.float32)
    cfr = np.ascontiguousarray(
        np.concatenate([bv_b, mr_h, mr_h], axis=1)).astype(np.float32)
    in_maps = []
    for i in range(NCORES):
        b, hf = i // 2, i % 2
        xs = np.zeros((TH, H), np.float32)
        if hf == 0:
            xs[HALO:] = states[b, 0:TC]
        else:
            xs[:] = states[b, TC - HALO: 2 * TC]
        x_h = np.ascontiguousarray(xs.T).astype(BF)   # [H, TH]
        x_f = x_h.astype(np.float32)
        u_h = (wt_h @ x_f).astype(BF)
        uo_h = np.zeros((1, 128 + TH), BF)
        uo_h[0, 0:128] = np.ones(128, BF)
        uo_h[0, 128:] = u_h
        yh_h = (a_f.T @ x_f[:, :HALO]).astype(BF)      # [H, 8]
        vtail_h = (x_f[:, TC:].T @ wv_f + bv).astype(BF)  # [8, H]
        in_maps.append({
            "x": x_h, "a": at_p, "wv": wv_p,
            "cf": (cf0 if hf == 0 else cfr), "ident": id_h,
            "uo": uo_h, "yhalo": yh_h, "vtail": vtail_h,
        })
    return in_maps


def kernel(states, Wq, bq, Wk, bk, Wv, bv, window):
    assert int(window) == HALO
    states = np.asarray(states, np.float32)
    nc = _build()
    hosts = _host_inputs(states, Wq, bq, Wk, bk, Wv, bv)
    in_maps = _shard_maps(states, hosts)
    res = run_bass_kernel_spmd(nc, in_maps, list(range(NCORES)))
    out = np.empty((B, T, H), np.float32)
    for i in range(NCORES):
        b, hf = i // 2, i % 2
        out[b, hf * TC:(hf + 1) * TC] = res.results[i]["out"].astype(np.float32)
    return out


# revision 15
# speedup vs baseline: 1.0238x; 1.0238x over previous
"""Local causal (sliding-window) attention kernel for Trainium2, SPMD over 8 cores.

Problem: states [4, 4096, 1024] f32; q/k/v = states @ W*.T + b*; each query t
attends keys t-8..t (window=8), softmax over valid positions, out = attn @ v.

Sharding: data-parallel, 8 shards = 4 batches x 2 sequence halves (2048 queries
each). The host supplies each shard's states pre-transposed to [H, 2056] with an
8-row halo (zero-padded at sequence start; masked out via the additive mask).

Score reformulation (saves one full GEMM): q.k = x_t^T A x_k + g.x_t + w.x_k
+ c0 with A = (Wq/sqrt(H))^T Wk precomputed on host. The device computes
Y = A @ X (one GEMM) instead of both Q and K projections; X itself serves as
the score lhsT. The rank-1 term u[k] = w.x_k is a host GEMV accumulated into
the score PSUM via a tiny K=1 matmul; per-query terms/constants cancel in
softmax, so no max-subtraction is needed (scores are O(1), masked lanes get
-30000 and underflow to exact zero in exp).

Scheduling notes (from perfetto traces): there is a ~7us fixed program
preamble before the first DMA descriptor can issue, each DMA descriptor
costs ~630ns on its issuing queue, and completion semaphores batch per
queue, so the input phase is descriptor-issue-bound, not bandwidth-bound.
Hence: few large DMAs (consts packed, Wv/A as single wide tiles loaded in 2
DMAs), issues spread over 4 engine queues, warm-up matmuls on a memset tile
to keep the PE HAM-warm through the load phase, V-GEMM groups emitted before
Y so the earliest-landing data is consumed first, bf16 output DMA.
"""

import numpy as np
import ml_dtypes

import concourse.bacc as bacc
import concourse.mybir as mybir
import concourse.tile as tile
from concourse.bass_utils import run_bass_kernel_spmd

B, T, H = 4, 4096, 1024
NCORES = 8
TC = T // 2            # queries per core
HALO = 8               # window size
TH = TC + HALO         # shard cols incl. halo
SPAN = 128 + HALO      # key span per 128-query tile
NT = TC // 128         # query tiles per core
HC = H // 128          # 128-row chunks of H
F32 = mybir.dt.float32
BF16 = mybir.dt.bfloat16
BF = ml_dtypes.bfloat16
AF = mybir.ActivationFunctionType
NWARM = 12             # HAM warm-up matmuls during the DMA load phase

_cache = {}


def _emit(nc, tc, aps, pools):
    (x_d, a_d, wv_d, cf_d, cb_d, uo_d, yh_d, vt_d, out_d) = aps
    consts, xw, acts, psP, psS, psT, psO, attn = pools

    # packed f32 consts: [bv | m0 | mr]
    cf = consts.tile([128, H + 2 * SPAN], F32, tag="cf", name="cf")
    bv_t = cf[:, 0:H]
    m0_t = cf[:, H:H + SPAN]
    mr_t = cf[:, H + SPAN:H + 2 * SPAN]
    id_t = consts.tile([128, 128], BF16, tag="id", name="id_t")
    uo = consts.tile([1, 128 + TH], BF16, tag="uo", name="uo")
    on_t = uo[:, 0:128]
    u_t = uo[:, 128:128 + TH]
    warm = consts.tile([128, 512], BF16, tag="warm", name="warm")

    xt = [xw.tile([128, TH], BF16, tag=f"x{c}", name=f"x{c}") for c in range(HC)]
    a_all = xw.tile([128, HC * H], BF16, tag="a_all", name="a_all")
    wv_all = xw.tile([128, HC * H], BF16, tag="wv_all", name="wv_all")
    at = [a_all[:, c * H:(c + 1) * H] for c in range(HC)]
    wvt = [wv_all[:, c * H:(c + 1) * H] for c in range(HC)]
    yt = [acts.tile([128, TH], BF16, tag=f"y{c}", name=f"y{c}") for c in range(HC)]
    vt = [acts.tile([128, H], BF16, tag=f"v{j}", name=f"v{j}")
          for j in range(NT)]
    vtail = acts.tile([HALO, H], BF16, tag="vtail", name="vtail")

    # --- DMA issue plan. Only sync/scalar have hardware DGE rings (gpsimd
    # DMAs go through slow software rings); each ~630ns DIRECT2D issue maps
    # round-robin onto 8 HW rings of ~100GB/s each, and issues block once a
    # ring's descriptor window fills. So: bulk tensors split into quarters
    # alternating between the two queues, in consumption order.
    nc.gpsimd.memset(warm[:], 1.0)
    nc.scalar.dma_start(cf[:], cf_d[:])
    nc.scalar.dma_start(id_t[:], cb_d[:])
    nc.scalar.dma_start(uo[:], uo_d[:])
    # wv first (V group 0 consumes it first), quarters on both queues
    for q in range(4):
        (nc.sync if q % 2 == 0 else nc.scalar).dma_start(
            wv_all[:, q * 2 * H:(q + 1) * 2 * H],
            wv_d[:, q * 2 * H:(q + 1) * 2 * H])
    # x segment 0 (cols 0:520) per chunk, alternating queues
    for c in range(HC):
        (nc.scalar if c % 2 else nc.sync).dma_start(
            xt[c][:, 0:520], x_d[c * 128:(c + 1) * 128, 0:520])
    # A quarters
    for q in range(4):
        (nc.sync if q % 2 == 0 else nc.scalar).dma_start(
            a_all[:, q * 2 * H:(q + 1) * 2 * H],
            a_d[:, q * 2 * H:(q + 1) * 2 * H])
    # Y halo cols (staged, scattered into yt by 8 small vector copies)
    yh_st = consts.tile([128, HC * HALO], BF16, tag="yhst", name="yh_st")
    nc.scalar.dma_start(yh_st[:], yh_d[:])
    nc.scalar.dma_start(vtail[:], vt_d[:])
    for c in range(HC):
        nc.vector.tensor_copy(yt[c][:, 0:HALO],
                              yh_st[:, c * HALO:(c + 1) * HALO])
    # remaining x segments
    for seg in range(3):
        lo = 520 + seg * 512
        for c in range(HC):
            (nc.sync if c % 2 else nc.scalar).dma_start(
                xt[c][:, lo:lo + 512], x_d[c * 128:(c + 1) * 128, lo:lo + 512])

    # --- HAM warm-up: junk matmuls on the memset tile fill the PE during
    # the load phase so real matmuls start at full clock.
    for w in range(NWARM):
        ps = psP.tile([128, 512], F32, tag="ps", name="pswarm")
        nc.tensor.matmul(ps[:], warm[:, 0:128], warm[:], start=True, stop=True)

    def emit_y(t4):
        off = HALO + t4 * 512
        for hc in range(HC):
            ps = psP.tile([128, 512], F32, tag="ps", name="psy")
            for c in range(HC):
                nc.tensor.matmul(
                    ps[:], at[c][:, hc * 128:(hc + 1) * 128],
                    xt[c][:, off: off + 512],
                    start=(c == 0), stop=(c == HC - 1))
            # alternate copy engine to balance Scalar/Vector queues
            if hc % 2 == 0:
                nc.scalar.copy(yt[hc][:, off: off + 512], ps[:])
            else:
                nc.vector.tensor_copy(yt[hc][:, off: off + 512], ps[:])

    def emit_v(j):
        for hh in range(2):
            ps = psP.tile([128, 512], F32, tag="ps", name="psv")
            for c in range(HC):
                nc.tensor.matmul(
                    ps[:], xt[c][:, j * 128: (j + 1) * 128],
                    wvt[c][:, hh * 512:(hh + 1) * 512],
                    start=(c == 0), stop=(c == HC - 1))
            nc.vector.tensor_add(
                vt[j][:, hh * 512:(hh + 1) * 512], ps[:],
                bv_t[:, hh * 512:(hh + 1) * 512])

    def emit_attn(j):
        s_ps = psS.tile([128, SPAN], F32, tag="s", name="s_ps")
        # rank-1 u term first: its LDWEIGHTS prefetches under prior streams
        nc.tensor.matmul(s_ps[:], on_t[:, 0:128],
                         u_t[:, j * 128: j * 128 + SPAN],
                         start=True, stop=False)
        for c in range(HC):
            nc.tensor.matmul(
                s_ps[:], xt[c][:, HALO + j * 128: HALO + (j + 1) * 128],
                yt[c][:, j * 128: j * 128 + SPAN],
                start=False, stop=(c == HC - 1))
        s_sb = attn.tile([128, SPAN], F32, tag="ssb", name="s_sb")
        nc.vector.tensor_add(s_sb[:], s_ps[:],
                             (m0_t if j == 0 else mr_t)[:])
        p_bf = attn.tile([128, SPAN], BF16, tag="p", name="p_bf")
        rowsum = attn.tile([128, 1], F32, tag="rs", name="rowsum")
        nc.scalar.activation(p_bf[:], s_sb[:], AF.Exp,
                             bias=0.0, scale=1.0,
                             accum_out=rowsum[:])
        rinv = attn.tile([128, 1], F32, tag="ri", name="rinv")
        nc.vector.reciprocal(rinv[:], rowsum[:])

        pt_ps = psT.tile([128, 256], BF16, tag="pt", name="pt_ps")
        nc.tensor.transpose(pt_ps[:, 0:128], p_bf[:, 0:128], id_t[:])
        nc.tensor.transpose(pt_ps[:HALO, 128:256], p_bf[:, 128:SPAN], id_t[:])
        ptab = attn.tile([128, 256], BF16, tag="ptab", name="ptab")
        nc.scalar.copy(ptab[:], pt_ps[:])

        vnext = vtail if j == NT - 1 else vt[j + 1]
        out_sb = attn.tile([128, H], BF16, tag="osb", name="out_sb")
        for hh in range(2):
            o_ps = psO.tile([128, 512], F32, tag="o", name="o_ps")
            nc.tensor.matmul(o_ps[:], ptab[:, 0:128],
                             vt[j][:, hh * 512:(hh + 1) * 512],
                             start=True, stop=False)
            nc.tensor.matmul(o_ps[:], ptab[:HALO, 128:256],
                             vnext[:HALO, hh * 512:(hh + 1) * 512],
                             start=False, stop=True)
            nc.scalar.activation(
                out_sb[:, hh * 512:(hh + 1) * 512], o_ps[:],
                AF.Copy, bias=0.0, scale=rinv[:])
        nc.sync.dma_start(out_d[j * 128:(j + 1) * 128, :], out_sb[:])

    # Interleave: V group first (its data lands first), then Y segment,
    # then attention tiles whose span fits the Y columns produced so far.
    # attn j needs Y cols < 8+512*(t4+1) and vt[j+1] (so attn(4*t4-1) waits
    # for the following V group).
    for t4 in range(TC // 512):
        for j in range(4 * t4, 4 * t4 + 4):
            emit_v(j)
        if t4 > 0:
            emit_attn(4 * t4 - 1)
        emit_y(t4)
        for j in range(4 * t4, 4 * t4 + 3):
            emit_attn(j)
    emit_attn(NT - 1)


def _build(loop_reps=None, trace_sim=False):
    key = ("nc", loop_reps, trace_sim)
    if key in _cache:
        return _cache[key]
    nc = bacc.Bacc("TRN2", target_bir_lowering=False, debug=False,
                   num_devices=NCORES)

    aps = (
        nc.dram_tensor("x", [H, TH], BF16, kind="ExternalInput").ap(),
        nc.dram_tensor("a", [128, HC * H], BF16, kind="ExternalInput").ap(),
        nc.dram_tensor("wv", [128, HC * H], BF16, kind="ExternalInput").ap(),
        nc.dram_tensor("cf", [128, H + 2 * SPAN], F32,
                       kind="ExternalInput").ap(),
        nc.dram_tensor("ident", [128, 128], BF16, kind="ExternalInput").ap(),
        nc.dram_tensor("uo", [1, 128 + TH], BF16, kind="ExternalInput").ap(),
        nc.dram_tensor("yhalo", [128, HC * HALO], BF16,
                       kind="ExternalInput").ap(),
        nc.dram_tensor("vtail", [HALO, H], BF16, kind="ExternalInput").ap(),
        nc.dram_tensor("out", [TC, H], BF16, kind="ExternalOutput").ap(),
    )

    with tile.TileContext(nc, trace_sim=trace_sim) as tc:
        with (
            tc.tile_pool(name="consts", bufs=1) as consts,
            tc.tile_pool(name="xw", bufs=1) as xw,
            tc.tile_pool(name="acts", bufs=1) as acts,
            tc.tile_pool(name="psP", bufs=2, space="PSUM") as psP,
            tc.tile_pool(name="psS", bufs=2, space="PSUM") as psS,
            tc.tile_pool(name="psT", bufs=2, space="PSUM") as psT,
            tc.tile_pool(name="psO", bufs=2, space="PSUM") as psO,
            tc.tile_pool(name="attn", bufs=3) as attn,
        ):
            pools = (consts, xw, acts, psP, psS, psT, psO, attn)
            if loop_reps:
                with tc.For_i(0, loop_reps, 1):
                    _emit(nc, tc, aps, pools)
            else:
                _emit(nc, tc, aps, pools)

    nc.compile()
    _cache[key] = nc
    return nc


def _host_inputs(states, Wq, bq, Wk, bk, Wv, bv):
    """Shared (per-run) host-side tensor prep."""
    scale = 1.0 / np.sqrt(H)
    Wq = np.asarray(Wq, np.float32)
    Wk = np.asarray(Wk, np.float32)
    Wv = np.asarray(Wv, np.float32)
    bq = np.asarray(bq, np.float32)
    bk = np.asarray(bk, np.float32)
    bv = np.asarray(bv, np.float32)
    Wqs = Wq * scale
    # A = Wqs.T @ Wk ; device lhsT layout needs A.T = Wk.T @ Wqs
    at_h = np.ascontiguousarray(Wk.T @ Wqs).astype(BF)
    # per-key rank-1 vector; per-query term and constants cancel in softmax
    wt_h = Wk.T @ (bq * scale)
    wv_h = np.ascontiguousarray(Wv.T).astype(BF)
    # chunk-packed [128, 8*H] device layouts: block c = rows c*128..c*128+127
    at_p = np.ascontiguousarray(
        at_h.reshape(HC, 128, H).transpose(1, 0, 2).reshape(128, HC * H))
    wv_p = np.ascontiguousarray(
        wv_h.reshape(HC, 128, H).transpose(1, 0, 2).reshape(128, HC * H))
    m = np.arange(128)[:, None]
    n = np.arange(SPAN)[None, :]
    band = (n >= m) & (n <= m + HALO)
    mr_h = np.where(band, 0.0, -30000.0).astype(np.float32)
    m0_h = np.where(band & (n >= HALO), 0.0, -30000.0).astype(np.float32)
    id_h = np.eye(128).astype(BF)
    return at_h, at_p, wt_h, wv_h, wv_p, m0_h, mr_h, id_h, bv


def _shard_maps(states, hosts):
    at_h, at_p, wt_h, wv_h, wv_p, m0_h, mr_h, id_h, bv = hosts
    a_f = at_h.astype(np.float32)      # [hin, hout] = A.T in bf16 precision
    wv_f = wv_h.astype(np.float32)
    bv_b = np.broadcast_to(bv, (128, H))
    cf0 = np.ascontiguousarray(
        np.concatenate([bv_b, m0_h, mr_h], axis=1)).astype(np.float32)
    cfr = np.ascontiguousarray(
        np.concatenate([bv_b, mr_h, mr_h], axis=1)).astype(np.float32)
    in_maps = []
    for i in range(NCORES):
        b, hf = i // 2, i % 2
        xs = np.zeros((TH, H), np.float32)
        if hf == 0:
            xs[HALO:] = states[b, 0:TC]
        else:
            xs[:] = states[b, TC - HALO: 2 * TC]
        x_h = np.ascontiguousarray(xs.T).astype(BF)   # [H, TH]
        x_f = x_h.astype(np.float32)
        u_h = (wt_h @ x_f).astype(BF)
        uo_h = np.zeros((1, 128 + TH), BF)
        uo_h[0, 0:128] = np.ones(128, BF)
        uo_h[0, 128:] = u_h
        yh_h = (a_f.T @ x_f[:, :HALO]).astype(BF)      # [H, 8]
        yh_h = np.ascontiguousarray(                   # chunk-packed [128, 64]
            yh_h.reshape(HC, 128, HALO).transpose(1, 0, 2).reshape(128, -1))
        vtail_h = (x_f[:, TC:].T @ wv_f + bv).astype(BF)  # [8, H]
        in_maps.append({
            "x": x_h, "a": at_p, "wv": wv_p,
            "cf": (cf0 if hf == 0 else cfr), "ident": id_h,
            "uo": uo_h, "yhalo": yh_h, "vtail": vtail_h,
        })
    return in_maps


def kernel(states, Wq, bq, Wk, bk, Wv, bv, window):
    assert int(window) == HALO
    states = np.asarray(states, np.float32)
    nc = _build()
    hosts = _host_inputs(states, Wq, bq, Wk, bk, Wv, bv)
    in_maps = _shard_maps(states, hosts)
    res = run_bass_kernel_spmd(nc, in_maps, list(range(NCORES)))
    out = np.empty((B, T, H), np.float32)
    for i in range(NCORES):
        b, hf = i // 2, i % 2
        out[b, hf * TC:(hf + 1) * TC] = res.results[i]["out"].astype(np.float32)
    return out


# revision 16
# speedup vs baseline: 1.0606x; 1.0360x over previous
"""Local causal (sliding-window) attention kernel for Trainium2, SPMD over 8 cores.

Problem: states [4, 4096, 1024] f32; q/k/v = states @ W*.T + b*; each query t
attends keys t-8..t (window=8), softmax over valid positions, out = attn @ v.

Sharding: data-parallel, 8 shards = 4 batches x 2 sequence halves (2048 queries
each). The host supplies each shard's states pre-transposed to [H, 2056] with an
8-col halo (zero-padded at sequence start; masked out via the additive mask).

Score reformulation (saves one full GEMM): q.k = x_t^T A x_k + g.x_t + w.x_k
+ c0 with A = (Wq/sqrt(H))^T Wk precomputed on host. The device computes the
query-side projection Q' = A^T @ X (one GEMM) instead of both Q and K
projections; X itself serves as the score rhs, so the key side needs no halo
projection at all. The rank-1 term u[k] = w.x_k is a host GEMV shipped
partition-broadcast and added on DVE together with the band mask; per-query
terms/constants cancel in softmax, so no max-subtraction is needed (scores
are O(1), masked lanes get -30000 and underflow to exact zero in exp).

Scheduling notes (from perfetto traces): ~7us fixed program preamble before
the first DMA descriptor issues; each DIRECT2D issue costs ~630ns on its
queue; only sync/scalar have hardware DGE rings (8 rings, ~100GB/s each,
FIFO per ring), gpsimd DMAs take slow software rings. So: bulk tensors go
out as ~256KB per-chunk descriptors alternating between the sync and scalar
queues in consumption order, consts packed into one bf16 descriptor, and
warm-up matmuls on a memset tile keep the PE HAM-warm through the load
phase. V-GEMM groups are emitted before the Q' segment so the earliest
landing data is consumed first; output is DMA'd as bf16 halves (host
upcasts to f32).
"""

import numpy as np
import ml_dtypes

import concourse.bacc as bacc
import concourse.mybir as mybir
import concourse.tile as tile
from concourse.bass_utils import run_bass_kernel_spmd

B, T, H = 4, 4096, 1024
NCORES = 8
TC = T // 2            # queries per core
HALO = 8               # window size
TH = TC + HALO         # shard cols incl. halo
SPAN = 128 + HALO      # key span per 128-query tile
NT = TC // 128         # query tiles per core
HC = H // 128          # 128-row chunks of H
F32 = mybir.dt.float32
BF16 = mybir.dt.bfloat16
BF = ml_dtypes.bfloat16
AF = mybir.ActivationFunctionType
NWARM = 16             # HAM warm-up matmuls during the DMA load phase
CB_W = 128 + H + 2 * SPAN   # packed consts: [id | bv | m0 | mr]

_cache = {}


def _emit(nc, tc, aps, pools):
    (x_d, a_d, wv_d, cb_d, ub_d, vt_d, out_d) = aps
    consts, xw, acts, psP, psS, psT, psO, attn = pools

    cb = consts.tile([128, CB_W], BF16, tag="cb", name="cb")
    id_t = cb[:, 0:128]
    bv_t = cb[:, 128:128 + H]
    m0_t = cb[:, 128 + H:128 + H + SPAN]
    mr_t = cb[:, 128 + H + SPAN:CB_W]
    ub = consts.tile([128, TH], BF16, tag="ub", name="ub")
    warm = consts.tile([128, 512], BF16, tag="warm", name="warm")

    xt = [xw.tile([128, TH], BF16, tag=f"x{c}", name=f"x{c}") for c in range(HC)]
    a_all = xw.tile([128, HC * H], BF16, tag="a_all", name="a_all")
    wv_all = xw.tile([128, HC * H], BF16, tag="wv_all", name="wv_all")
    at = [a_all[:, c * H:(c + 1) * H] for c in range(HC)]
    wvt = [wv_all[:, c * H:(c + 1) * H] for c in range(HC)]
    qt = [acts.tile([128, TC], BF16, tag=f"q{c}", name=f"q{c}")
          for c in range(HC)]
    vt = [acts.tile([128, H], BF16, tag=f"v{j}", name=f"v{j}")
          for j in range(NT)]
    vtail = acts.tile([HALO, H], BF16, tag="vtail", name="vtail")

    # --- DMA issue plan: sync/scalar HW queues only, ~256KB descriptors in
    # consumption order, alternating queues so consecutive tensors land on
    # different DMA rings.
    nc.gpsimd.memset(warm[:], 1.0)
    nc.scalar.dma_start(cb[:], cb_d[:])
    for c in range(HC):       # wv first: V group 0 consumes it first
        (nc.sync if c % 2 else nc.scalar).dma_start(
            wv_all[:, c * H:(c + 1) * H], wv_d[:, c * H:(c + 1) * H])
    for c in range(HC):       # x cols 0:520 (V groups 0, Q' seg 0, attn 0-3)
        (nc.scalar if c % 2 else nc.sync).dma_start(
            xt[c][:, 0:520], x_d[c * 128:(c + 1) * 128, 0:520])
    for c in range(HC):       # A chunks
        (nc.sync if c % 2 else nc.scalar).dma_start(
            a_all[:, c * H:(c + 1) * H], a_d[:, c * H:(c + 1) * H])
    nc.scalar.dma_start(ub[:], ub_d[:])
    nc.scalar.dma_start(vtail[:], vt_d[:])
    for seg in range(3):      # remaining x column segments
        lo = 520 + seg * 512
        for c in range(HC):
            (nc.sync if c % 2 else nc.scalar).dma_start(
                xt[c][:, lo:lo + 512], x_d[c * 128:(c + 1) * 128, lo:lo + 512])

    # --- HAM warm-up: junk matmuls on the memset tile fill the PE during
    # the load phase so real matmuls start at full clock.
    for w in range(NWARM):
        ps = psP.tile([128, 512], F32, tag="ps", name="pswarm")
        nc.tensor.matmul(ps[:], warm[:, 0:128], warm[:], start=True, stop=True)

    def emit_q(t4):
        off = t4 * 512
        for hc in range(HC):
            ps = psP.tile([128, 512], F32, tag="ps", name="psq")
            for c in range(HC):
                nc.tensor.matmul(
                    ps[:], at[c][:, hc * 128:(hc + 1) * 128],
                    xt[c][:, HALO + off: HALO + off + 512],
                    start=(c == 0), stop=(c == HC - 1))
            # alternate copy engine to balance Scalar/Vector queues
            if hc % 2 == 0:
                nc.scalar.copy(qt[hc][:, off: off + 512], ps[:])
            else:
                nc.vector.tensor_copy(qt[hc][:, off: off + 512], ps[:])

    def emit_v(j):
        for hh in range(2):
            ps = psP.tile([128, 512], F32, tag="ps", name="psv")
            for c in range(HC):
                nc.tensor.matmul(
                    ps[:], xt[c][:, j * 128: (j + 1) * 128],
                    wvt[c][:, hh * 512:(hh + 1) * 512],
                    start=(c == 0), stop=(c == HC - 1))
            nc.vector.tensor_add(
                vt[j][:, hh * 512:(hh + 1) * 512], ps[:],
                bv_t[:, hh * 512:(hh + 1) * 512])

    def emit_attn(j):
        s_ps = psS.tile([128, SPAN], F32, tag="s", name="s_ps")
        for c in range(HC):
            nc.tensor.matmul(
                s_ps[:], qt[c][:, j * 128: (j + 1) * 128],
                xt[c][:, j * 128: j * 128 + SPAN],
                start=(c == 0), stop=(c == HC - 1))
        s_sb = attn.tile([128, SPAN], F32, tag="ssb", name="s_sb")
        nc.vector.tensor_add(s_sb[:], s_ps[:],
                             (m0_t if j == 0 else mr_t)[:])
        nc.vector.tensor_add(s_sb[:], s_sb[:],
                             ub[:, j * 128: j * 128 + SPAN])
        p_bf = attn.tile([128, SPAN], BF16, tag="p", name="p_bf")
        rowsum = attn.tile([128, 1], F32, tag="rs", name="rowsum")
        nc.scalar.activation(p_bf[:], s_sb[:], AF.Exp,
                             bias=0.0, scale=1.0,
                             accum_out=rowsum[:])
        rinv = attn.tile([128, 1], F32, tag="ri", name="rinv")
        nc.vector.reciprocal(rinv[:], rowsum[:])

        pt_ps = psT.tile([128, 256], BF16, tag="pt", name="pt_ps")
        nc.tensor.transpose(pt_ps[:, 0:128], p_bf[:, 0:128], id_t[:])
        nc.tensor.transpose(pt_ps[:HALO, 128:256], p_bf[:, 128:SPAN], id_t[:])
        ptab = attn.tile([128, 256], BF16, tag="ptab", name="ptab")
        nc.scalar.copy(ptab[:], pt_ps[:])

        vnext = vtail if j == NT - 1 else vt[j + 1]
        out_sb = attn.tile([128, H], BF16, tag="osb", name="out_sb")
        for hh in range(2):
            o_ps = psO.tile([128, 512], F32, tag="o", name="o_ps")
            nc.tensor.matmul(o_ps[:], ptab[:, 0:128],
                             vt[j][:, hh * 512:(hh + 1) * 512],
                             start=True, stop=False)
            nc.tensor.matmul(o_ps[:], ptab[:HALO, 128:256],
                             vnext[:HALO, hh * 512:(hh + 1) * 512],
                             start=False, stop=True)
            nc.scalar.activation(
                out_sb[:, hh * 512:(hh + 1) * 512], o_ps[:],
                AF.Copy, bias=0.0, scale=rinv[:])
            nc.sync.dma_start(
                out_d[j * 128:(j + 1) * 128, hh * 512:(hh + 1) * 512],
                out_sb[:, hh * 512:(hh + 1) * 512])

    # Interleave: V group first (its data lands first), then Q' segment,
    # then attention tiles whose queries fit the Q' columns produced so far.
    # attn j needs Q' cols < 512*(t4+1) and vt[j+1] (so attn(4*t4-1) waits
    # for the following V group).
    for t4 in range(TC // 512):
        for j in range(4 * t4, 4 * t4 + 4):
            emit_v(j)
        if t4 > 0:
            emit_attn(4 * t4 - 1)
        emit_q(t4)
        for j in range(4 * t4, 4 * t4 + 3):
            emit_attn(j)
    emit_attn(NT - 1)


def _build(loop_reps=None, trace_sim=False):
    key = ("nc", loop_reps, trace_sim)
    if key in _cache:
        return _cache[key]
    nc = bacc.Bacc("TRN2", target_bir_lowering=False, debug=False,
                   num_devices=NCORES)

    aps = (
        nc.dram_tensor("x", [H, TH], BF16, kind="ExternalInput").ap(),
        nc.dram_tensor("a", [128, HC * H], BF16, kind="ExternalInput").ap(),
        nc.dram_tensor("wv", [128, HC * H], BF16, kind="ExternalInput").ap(),
        nc.dram_tensor("cb", [128, CB_W], BF16, kind="ExternalInput").ap(),
        nc.dram_tensor("ub", [128, TH], BF16, kind="ExternalInput").ap(),
        nc.dram_tensor("vtail", [HALO, H], BF16, kind="ExternalInput").ap(),
        nc.dram_tensor("out", [TC, H], BF16, kind="ExternalOutput").ap(),
    )

    with tile.TileContext(nc, trace_sim=trace_sim) as tc:
        with (
            tc.tile_pool(name="consts", bufs=1) as consts,
            tc.tile_pool(name="xw", bufs=1) as xw,
            tc.tile_pool(name="acts", bufs=1) as acts,
            tc.tile_pool(name="psP", bufs=2, space="PSUM") as psP,
            tc.tile_pool(name="psS", bufs=2, space="PSUM") as psS,
            tc.tile_pool(name="psT", bufs=2, space="PSUM") as psT,
            tc.tile_pool(name="psO", bufs=2, space="PSUM") as psO,
            tc.tile_pool(name="attn", bufs=3) as attn,
        ):
            pools = (consts, xw, acts, psP, psS, psT, psO, attn)
            if loop_reps:
                with tc.For_i(0, loop_reps, 1):
                    _emit(nc, tc, aps, pools)
            else:
                _emit(nc, tc, aps, pools)

    nc.compile()
    _cache[key] = nc
    return nc


def _host_inputs(states, Wq, bq, Wk, bk, Wv, bv):
    """Shared (per-run) host-side tensor prep."""
    scale = 1.0 / np.sqrt(H)
    Wq = np.asarray(Wq, np.float32)
    Wk = np.asarray(Wk, np.float32)
    Wv = np.asarray(Wv, np.float32)
    bq = np.asarray(bq, np.float32)
    bv = np.asarray(bv, np.float32)
    Wqs = Wq * scale
    # S = x_q^T A x_k with A = Wqs^T Wk; Q' = A^T X needs lhsT = A chunks
    a_h = np.ascontiguousarray(Wqs.T @ Wk).astype(BF)
    wt_h = Wk.T @ (bq * scale)   # u[k] = wt . x_k
    wv_h = np.ascontiguousarray(Wv.T).astype(BF)
    # chunk-packed [128, 8*H] device layouts: block c = rows c*128..c*128+127
    a_p = np.ascontiguousarray(
        a_h.reshape(HC, 128, H).transpose(1, 0, 2).reshape(128, HC * H))
    wv_p = np.ascontiguousarray(
        wv_h.reshape(HC, 128, H).transpose(1, 0, 2).reshape(128, HC * H))
    m = np.arange(128)[:, None]
    n = np.arange(SPAN)[None, :]
    band = (n >= m) & (n <= m + HALO)
    mr_h = np.where(band, 0.0, -30000.0).astype(BF)
    m0_h = np.where(band & (n >= HALO), 0.0, -30000.0).astype(BF)
    id_h = np.eye(128).astype(BF)
    bv_b = np.broadcast_to(bv.astype(BF), (128, H))
    cb0 = np.ascontiguousarray(np.concatenate(
        [id_h, bv_b, m0_h, mr_h], axis=1))
    cbr = np.ascontiguousarray(np.concatenate(
        [id_h, bv_b, mr_h, mr_h], axis=1))
    return a_p, wt_h, wv_h, wv_p, cb0, cbr, bv


def _shard_maps(states, hosts):
    a_p, wt_h, wv_h, wv_p, cb0, cbr, bv = hosts
    wv_f = wv_h.astype(np.float32)
    in_maps = []
    for i in range(NCORES):
        b, hf = i // 2, i % 2
        xs = np.zeros((TH, H), np.float32)
        if hf == 0:
            xs[HALO:] = states[b, 0:TC]
        else:
            xs[:] = states[b, TC - HALO: 2 * TC]
        x_h = np.ascontiguousarray(xs.T).astype(BF)   # [H, TH]
        x_f = x_h.astype(np.float32)
        u_h = (wt_h @ x_f).astype(BF)                 # [TH]
        ub_h = np.ascontiguousarray(np.broadcast_to(u_h, (128, TH)))
        vtail_h = (x_f[:, TC:].T @ wv_f + bv).astype(BF)  # [8, H]
        in_maps.append({
            "x": x_h, "a": a_p, "wv": wv_p,
            "cb": (cb0 if hf == 0 else cbr),
            "ub": ub_h, "vtail": vtail_h,
        })
    return in_maps


def kernel(states, Wq, bq, Wk, bk, Wv, bv, window):
    assert int(window) == HALO
    states = np.asarray(states, np.float32)
    nc = _build()
    hosts = _host_inputs(states, Wq, bq, Wk, bk, Wv, bv)
    in_maps = _shard_maps(states, hosts)
    res = run_bass_kernel_spmd(nc, in_maps, list(range(NCORES)))
    out = np.empty((B, T, H), np.float32)
    for i in range(NCORES):
        b, hf = i // 2, i % 2
        out[b, hf * TC:(hf + 1) * TC] = res.results[i]["out"].astype(np.float32)
    return out
